# revision 1
# baseline (speedup 1.0000x reference)
"""Binary dense layer on 8 Trainium2 NeuronCores.

Computes out = sign(X) @ sign(K) + bias for X:[8192,2048] f32,
K:[2048,2048] f32, bias:[2048] f32 (sign(x) = +1 if x >= 0 else -1).

Strategy: data-parallel over the batch dim (1024 rows per core), K/bias
replicated. Per core the kernel computes outT = (sign(K).T @ sign(X_c.T))
so that both matmul operands have the contraction dim on partitions with
fully contiguous DMA loads (X is shipped host-transposed, K is shipped as
column panels).

Sign is computed exactly on the vector engine as (x >= 0) - 0.5 -> {-0.5,
+0.5} in bf16 (one op per element). Products are then +-0.25, accumulated
exactly in fp32 PSUM (|sum| <= 512), and the final activation copy applies
scale=4 and the per-partition bias, recovering the exact integer result.
"""

import sys

import numpy as np

_REPO = "/opt/trn_rl_repo"
if _REPO not in sys.path:
    sys.path.insert(0, _REPO)

N_CORES = 8
B, D, U = 8192, 2048, 2048
M = B // N_CORES      # batch rows per core (1024)
PT = 128              # partition tile
DT = D // PT          # contraction tiles (16)
NT = U // PT          # output-column tiles (16)
MCH = 512             # PSUM free-dim chunk
NM = M // MCH         # m-chunks per core (2)

TRACE = False
LAST_RESULT = None

_CACHE = {}


def _install_ntff_hook():
    """Make run_bass_kernel_spmd(trace=True) work when the image's antenv
    package lacks the axon_hooks shim. Profiling only; no effect on results."""
    import types

    try:
        import antenv.axon_hooks  # noqa: F401
        return True
    except ImportError:
        pass
    try:
        from trn_agent_boot.trn_boot import _ntff_profile_via_ctypes

        hook = _ntff_profile_via_ctypes("/opt/axon/libaxon_pjrt.so")
        if hook is None:
            return False
        mod = types.ModuleType("antenv.axon_hooks")
        state = {"hook": hook}
        mod.set_axon_ntff_profile_hook = lambda h: state.__setitem__("hook", h)
        mod.get_axon_ntff_profile_hook = lambda: state["hook"]
        sys.modules["antenv.axon_hooks"] = mod
        import antenv

        antenv.axon_hooks = mod
        return True
    except Exception:
        return False


def _build():
    import concourse.bacc as bacc
    import concourse.mybir as mybir
    import concourse.tile as tile

    f32 = mybir.dt.float32
    bf16 = mybir.dt.bfloat16
    Alu = mybir.AluOpType
    Act = mybir.ActivationFunctionType

    nc = bacc.Bacc("TRN2", target_bir_lowering=False, debug=False)
    xt = nc.dram_tensor("xt", [D, M], f32, kind="ExternalInput").ap()
    kp = nc.dram_tensor("kp", [NT, D, PT], f32, kind="ExternalInput").ap()
    bt = nc.dram_tensor("bt", [PT, NT], f32, kind="ExternalInput").ap()
    out = nc.dram_tensor("out", [U, M], f32, kind="ExternalOutput").ap()

    with tile.TileContext(nc) as tc:
        with (
            tc.tile_pool(name="xraw", bufs=3) as xraw_pool,
            tc.tile_pool(name="xsign", bufs=DT) as xsign_pool,
            tc.tile_pool(name="kraw", bufs=3) as kraw_pool,
            tc.tile_pool(name="ksign", bufs=4) as ksign_pool,
            tc.tile_pool(name="psum", bufs=8, space="PSUM") as psum_pool,
            tc.tile_pool(name="osb", bufs=3) as osb_pool,
            tc.tile_pool(name="bias", bufs=1) as bias_pool,
        ):
            bias_sb = bias_pool.tile([PT, NT], f32)
            nc.sync.dma_start(out=bias_sb[:], in_=bt[:])

            # X.T shard: load d-tiles (contiguous) and sign to +-0.5 bf16.
            xsign = []
            for d in range(DT):
                xr = xraw_pool.tile([PT, M], f32)
                nc.sync.dma_start(out=xr[:], in_=xt[d * PT:(d + 1) * PT, :])
                xs = xsign_pool.tile([PT, M], bf16)
                nc.vector.tensor_scalar(
                    out=xs[:], in0=xr[:], scalar1=0.0, scalar2=0.5,
                    op0=Alu.is_ge, op1=Alu.subtract)
                xsign.append(xs)

            # Stream K column panels; each panel is self-contained in d so
            # the matmul accumulation for its 128 output columns can finish
            # as soon as the panel (and X.T) is resident.
            for n in range(NT):
                kr = kraw_pool.tile([PT, DT, PT], f32)
                nc.sync.dma_start(
                    out=kr[:], in_=kp[n].rearrange("(i p) j -> p i j", p=PT))
                ks = ksign_pool.tile([PT, DT, PT], bf16)
                nc.vector.tensor_scalar(
                    out=ks[:], in0=kr[:], scalar1=0.0, scalar2=0.5,
                    op0=Alu.is_ge, op1=Alu.subtract)

                ot = osb_pool.tile([PT, M], f32)
                for m in range(NM):
                    ps = psum_pool.tile([PT, MCH], f32)
                    for d in range(DT):
                        nc.tensor.matmul(
                            ps[:],
                            ks[:, d, :],
                            xsign[d][:, m * MCH:(m + 1) * MCH],
                            start=(d == 0),
                            stop=(d == DT - 1),
                        )
                    nc.scalar.activation(
                        ot[:, m * MCH:(m + 1) * MCH], ps[:], Act.Identity,
                        bias=bias_sb[:, n:n + 1], scale=4.0)
                nc.sync.dma_start(out=out[n * PT:(n + 1) * PT, :], in_=ot[:])

    nc.compile()
    return nc


def kernel(**inputs):
    x = np.ascontiguousarray(np.asarray(inputs["inputs"], dtype=np.float32))
    k = np.ascontiguousarray(np.asarray(inputs["kernel"], dtype=np.float32))
    b = np.ascontiguousarray(np.asarray(inputs["bias"], dtype=np.float32))
    assert x.shape == (B, D) and k.shape == (D, U) and b.shape == (U,)

    from concourse.bass_utils import run_bass_kernel_spmd

    if TRACE:
        _install_ntff_hook()

    if "nc" not in _CACHE:
        _CACHE["nc"] = _build()
    nc = _CACHE["nc"]

    xt_full = np.ascontiguousarray(x.T)                                 # [D, B]
    kp = np.ascontiguousarray(k.reshape(D, NT, PT).transpose(1, 0, 2))  # [NT, D, PT]
    bt = np.ascontiguousarray(b.reshape(NT, PT).T)                      # [PT, NT]

    in_maps = []
    for c in range(N_CORES):
        xt_c = np.ascontiguousarray(xt_full[:, c * M:(c + 1) * M])
        in_maps.append({"xt": xt_c, "kp": kp, "bt": bt})

    global LAST_RESULT
    res = run_bass_kernel_spmd(nc, in_maps, list(range(N_CORES)), trace=TRACE)
    LAST_RESULT = res

    outs = [np.asarray(r["out"]) for r in res.results]
    full = np.concatenate([o.T for o in outs], axis=0)
    return np.ascontiguousarray(full).astype(np.float32)


# revision 4
# speedup vs baseline: 1.0666x; 1.0666x over previous
"""Binary dense layer on 8 Trainium2 NeuronCores.

Computes out = sign(X) @ sign(K) + bias for X:[8192,2048] f32,
K:[2048,2048] f32, bias:[2048] f32 (sign(x) = +1 if x >= 0 else -1).

Strategy: data-parallel over the batch dim (1024 rows per core), K/bias
replicated. Per core the kernel computes outT = (sign(K).T @ sign(X_c.T))
so that both matmul operands have the contraction dim on partitions with
fully contiguous DMA loads (X is shipped host-transposed, K is shipped as
column panels).

Sign is computed exactly on the vector engine as (x >= 0) - 0.5 -> {-0.5,
+0.5} in bf16 (one op per element). Products are then +-0.25, accumulated
exactly in fp32 PSUM (|sum| <= 512), and the final activation copy applies
scale=4 and the per-partition bias, recovering the exact integer result.
"""

import sys

import numpy as np

_REPO = "/opt/trn_rl_repo"
if _REPO not in sys.path:
    sys.path.insert(0, _REPO)

N_CORES = 8
B, D, U = 8192, 2048, 2048
M = B // N_CORES      # batch rows per core (1024)
PT = 128              # partition tile
DT = D // PT          # contraction tiles (16)
NT = U // PT          # output-column tiles (16)
MCH = 512             # PSUM free-dim chunk
NM = M // MCH         # m-chunks per core (2)

TRACE = False
LAST_RESULT = None

_CACHE = {}


def _install_ntff_hook():
    """Make run_bass_kernel_spmd(trace=True) work when the image's antenv
    package lacks the axon_hooks shim. Profiling only; no effect on results."""
    import types

    try:
        import antenv.axon_hooks  # noqa: F401
        return True
    except ImportError:
        pass
    try:
        from trn_agent_boot.trn_boot import _ntff_profile_via_ctypes

        hook = _ntff_profile_via_ctypes("/opt/axon/libaxon_pjrt.so")
        if hook is None:
            return False
        mod = types.ModuleType("antenv.axon_hooks")
        state = {"hook": hook}
        mod.set_axon_ntff_profile_hook = lambda h: state.__setitem__("hook", h)
        mod.get_axon_ntff_profile_hook = lambda: state["hook"]
        sys.modules["antenv.axon_hooks"] = mod
        import antenv

        antenv.axon_hooks = mod
        return True
    except Exception:
        return False


def _build():
    import concourse.bacc as bacc
    import concourse.mybir as mybir
    import concourse.tile as tile

    f32 = mybir.dt.float32
    bf16 = mybir.dt.bfloat16
    Alu = mybir.AluOpType
    Act = mybir.ActivationFunctionType

    nc = bacc.Bacc("TRN2", target_bir_lowering=False, debug=False)
    xt = nc.dram_tensor("xt", [D, M], f32, kind="ExternalInput").ap()
    kp = nc.dram_tensor("kp", [NT, D, PT], f32, kind="ExternalInput").ap()
    bt = nc.dram_tensor("bt", [PT, NT], f32, kind="ExternalInput").ap()
    out = nc.dram_tensor("out", [U, M], f32, kind="ExternalOutput").ap()

    with tile.TileContext(nc) as tc:
        with (
            tc.tile_pool(name="xraw", bufs=4) as xraw_pool,
            tc.tile_pool(name="xsign", bufs=DT) as xsign_pool,
            tc.tile_pool(name="kraw", bufs=4) as kraw_pool,
            tc.tile_pool(name="ksign", bufs=6) as ksign_pool,
            tc.tile_pool(name="psum", bufs=8, space="PSUM") as psum_pool,
            tc.tile_pool(name="osb", bufs=3) as osb_pool,
            tc.tile_pool(name="bias", bufs=1) as bias_pool,
        ):
            # bias via SWDGE so the two HWDGE rings stay free for X/K.
            bias_sb = bias_pool.tile([PT, NT], f32)
            nc.gpsimd.dma_start(out=bias_sb[:], in_=bt[:])

            # X.T shard: load d-tiles (contiguous) and sign to +-0.5 bf16.
            xsign = []
            for d in range(DT):
                xr = xraw_pool.tile([PT, M], f32)
                nc.sync.dma_start(out=xr[:], in_=xt[d * PT:(d + 1) * PT, :])
                xs = xsign_pool.tile([PT, M], bf16)
                nc.vector.tensor_scalar(
                    out=xs[:], in0=xr[:], scalar1=0.0, scalar2=0.5,
                    op0=Alu.is_ge, op1=Alu.subtract)
                xsign.append(xs)

            # Stream K column panels; each panel is self-contained in d so
            # the matmul accumulation for its 128 output columns can finish
            # as soon as the panel (and X.T) is resident.
            # K panels go through the second HWDGE ring (ACT engine) so they
            # stream concurrently with the X.T loads on the sync ring.
            for n in range(NT):
                kr = kraw_pool.tile([PT, DT, PT], f32)
                nc.scalar.dma_start(
                    out=kr[:], in_=kp[n].rearrange("(i p) j -> p i j", p=PT))
                ks = ksign_pool.tile([PT, DT, PT], bf16)
                nc.vector.tensor_scalar(
                    out=ks[:], in0=kr[:], scalar1=0.0, scalar2=0.5,
                    op0=Alu.is_ge, op1=Alu.subtract)

                ot = osb_pool.tile([PT, M], f32)
                for m in range(NM):
                    ps = psum_pool.tile([PT, MCH], f32)
                    for d in range(DT):
                        nc.tensor.matmul(
                            ps[:],
                            ks[:, d, :],
                            xsign[d][:, m * MCH:(m + 1) * MCH],
                            start=(d == 0),
                            stop=(d == DT - 1),
                        )
                    nc.scalar.activation(
                        ot[:, m * MCH:(m + 1) * MCH], ps[:], Act.Identity,
                        bias=bias_sb[:, n:n + 1], scale=4.0)
                nc.gpsimd.dma_start(out=out[n * PT:(n + 1) * PT, :], in_=ot[:])

    nc.compile()
    return nc


def kernel(**inputs):
    x = np.ascontiguousarray(np.asarray(inputs["inputs"], dtype=np.float32))
    k = np.ascontiguousarray(np.asarray(inputs["kernel"], dtype=np.float32))
    b = np.ascontiguousarray(np.asarray(inputs["bias"], dtype=np.float32))
    assert x.shape == (B, D) and k.shape == (D, U) and b.shape == (U,)

    from concourse.bass_utils import run_bass_kernel_spmd

    if TRACE:
        _install_ntff_hook()

    if "nc" not in _CACHE:
        _CACHE["nc"] = _build()
    nc = _CACHE["nc"]

    xt_full = np.ascontiguousarray(x.T)                                 # [D, B]
    kp = np.ascontiguousarray(k.reshape(D, NT, PT).transpose(1, 0, 2))  # [NT, D, PT]
    bt = np.ascontiguousarray(b.reshape(NT, PT).T)                      # [PT, NT]

    in_maps = []
    for c in range(N_CORES):
        xt_c = np.ascontiguousarray(xt_full[:, c * M:(c + 1) * M])
        in_maps.append({"xt": xt_c, "kp": kp, "bt": bt})

    global LAST_RESULT
    res = run_bass_kernel_spmd(nc, in_maps, list(range(N_CORES)), trace=TRACE)
    LAST_RESULT = res

    outs = [np.asarray(r["out"]) for r in res.results]
    full = np.concatenate([o.T for o in outs], axis=0)
    return np.ascontiguousarray(full).astype(np.float32)
